# revision 15
# baseline (speedup 1.0000x reference)
"""BatchedGCN Trainium2 kernel (v3).

Per graph (batch element):
  norms_i = ||X_i||;  A = (X@X.T > 0.3*n_i*n_j) + I ; deg = rowsum(A); d = deg^-1/2
  H1 = relu(diag(d) A diag(d) (X @ W1.T) + b1)
  H2 = diag(d) A diag(d) (H1 @ W2.T) + b2
  out = H2 / max(||H2_row||, 1e-12)

(The cosine threshold is applied in un-normalized form:
 Xn_i . Xn_j > t  <=>  (X_i . X_j) * (1/max(n_i,eps)) / t > n_j — exact up
 to fp rounding; the diag(norm) factor relating X to Xn cancels against
 the un-normalized X used in the first linear layer.)

Sharding: data-parallel over B=32 across 8 cores (4 graphs each);
weights replicated.  Host-side layout prep: X is fed both natural and
transposed, pre-cast to bf16 (matmul compute dtype); W1^T/W2^T likewise.
All matmuls bf16 with fp32 PSUM accumulation; A is exact {0,1,2} bf16.
"""

from contextlib import ExitStack

import ml_dtypes
import numpy as np

import concourse.bass as bass
import concourse.mybir as mybir
import concourse.tile as tile
from concourse import bacc
from concourse.bass_utils import run_bass_kernel_spmd
from concourse.masks import make_identity

B, N, D_IN, D_H, D_OUT = 32, 1024, 768, 256, 128
N_CORES = 8
BPC = B // N_CORES          # graphs per core
NT = N // 128               # 8 row tiles
DTI = D_IN // 128           # 6 input-dim tiles
HC = D_H // 128             # 2 hidden chunks
F32 = mybir.dt.float32
BF16 = mybir.dt.bfloat16

KNN_THRESHOLD = 0.3
COS_EPS = 1e-8
NORM_EPS = 1e-12
ALU = mybir.AluOpType
AF = mybir.ActivationFunctionType


def build(n_batches: int = BPC):
    nc = bacc.Bacc("TRN2", debug=False, num_devices=N_CORES)
    Xb16 = nc.dram_tensor("Xb16", [n_batches, N, D_IN], BF16, kind="ExternalInput")
    XT = nc.dram_tensor("XT", [n_batches, D_IN, N], BF16, kind="ExternalInput")
    W1T = nc.dram_tensor("W1T", [D_IN, D_H], BF16, kind="ExternalInput")
    b1 = nc.dram_tensor("b1", [D_H], F32, kind="ExternalInput")
    W2T = nc.dram_tensor("W2T", [D_H, D_OUT], BF16, kind="ExternalInput")
    b2 = nc.dram_tensor("b2", [D_OUT], F32, kind="ExternalInput")
    Y = nc.dram_tensor("Y", [n_batches, N, D_OUT], F32, kind="ExternalOutput")
    with tile.TileContext(nc) as tc, ExitStack() as ctx:
        _body(ctx, tc, Xb16.ap(), XT.ap(), W1T.ap(), b1.ap(), W2T.ap(), b2.ap(),
              Y.ap(), n_batches)
    nc.compile()
    return nc


def _bcast_p(ap: bass.AP, parts: int = 128) -> bass.AP:
    """Broadcast a DRAM AP across `parts` partitions (partition-stride 0)."""
    return bass.AP(tensor=ap.tensor, offset=ap.offset, ap=[[0, parts]] + list(ap.ap))


def _body(ctx, tc, X, XT, W1T, b1, W2T, b2, Y, n_batches):
    nc = tc.nc

    singles = ctx.enter_context(tc.tile_pool(name="singles", bufs=1))
    xpool = ctx.enter_context(tc.tile_pool(name="xpool", bufs=3))
    sqj = ctx.enter_context(tc.tile_pool(name="sqj", bufs=2))
    xtpool = ctx.enter_context(tc.tile_pool(name="xtpool", bufs=2 * DTI))
    apool = ctx.enter_context(tc.tile_pool(name="apool", bufs=2 * NT))
    bvec = ctx.enter_context(tc.tile_pool(name="bvec", bufs=2))
    y1pool = ctx.enter_context(tc.tile_pool(name="y1pool", bufs=2 * NT))
    h1pool = ctx.enter_context(tc.tile_pool(name="h1pool", bufs=2 * HC))
    y2pool = ctx.enter_context(tc.tile_pool(name="y2pool", bufs=2 * NT))
    rppool = ctx.enter_context(tc.tile_pool(name="rppool", bufs=2))
    tmppool = ctx.enter_context(tc.tile_pool(name="tmppool", bufs=3))
    h2pool = ctx.enter_context(tc.tile_pool(name="h2pool", bufs=3))
    opool = ctx.enter_context(tc.tile_pool(name="opool", bufs=3))
    psA = ctx.enter_context(tc.tile_pool(name="psA", bufs=5, space="PSUM"))
    psB = ctx.enter_context(tc.tile_pool(name="psB", bufs=3, space="PSUM"))
    dramp = ctx.enter_context(tc.tile_pool(name="dramp", bufs=2, space="DRAM"))

    # ---- one-time constants (all plain loads, no prep chains) ---------------
    ident = singles.tile([128, 128], BF16)
    make_identity(nc, ident)

    b1col = singles.tile([128, HC], F32)
    nc.sync.dma_start(out=b1col, in_=bass.AP(tensor=b1.tensor, offset=b1.offset,
                                             ap=[[1, 128], [128, HC]]))
    b2rep = singles.tile([128, D_OUT], F32)
    nc.gpsimd.dma_start(out=b2rep, in_=_bcast_p(b2))

    w1t = []
    for dt in range(DTI):
        t = singles.tile([128, D_H], BF16, tag=f"w1t{dt}")
        nc.sync.dma_start(out=t, in_=W1T[dt * 128:(dt + 1) * 128, :])
        w1t.append(t)
    w2t = []
    for k in range(HC):
        t = singles.tile([128, D_OUT], BF16, tag=f"w2t{k}")
        nc.sync.dma_start(out=t, in_=W2T[k * 128:(k + 1) * 128, :])
        w2t.append(t)

    inv_t = 1.0 / KNN_THRESHOLD

    # ---- per-graph pipeline -------------------------------------------------
    for bi in range(n_batches):
        Xb = X[bi]
        XTb = XT[bi]
        Yb = Y[bi]

        # Phase A1 first: row norms from X natural layout — this feeds the
        # threshold bound (Nrep), the longest per-batch latency chain.
        ssqv = bvec.tile([128, NT], F32)
        for nt in range(NT):
            xf = xpool.tile([128, D_IN], BF16, tag="xf")
            nc.sync.dma_start(out=xf, in_=Xb[nt * 128:(nt + 1) * 128, :])
            sj = sqj.tile([128, D_IN], F32)
            nc.scalar.activation(out=sj, in_=xf, func=AF.Square,
                                 accum_out=ssqv[:, nt:nt + 1])
        ncol = bvec.tile([128, NT], F32)
        nc.scalar.sqrt(out=ncol, in_=ssqv)
        nclamp = bvec.tile([128, NT], F32)
        nc.vector.tensor_scalar_max(nclamp, ncol, COS_EPS)
        rcol = bvec.tile([128, NT], F32)
        nc.vector.reciprocal(out=rcol, in_=nclamp)
        rc03 = bvec.tile([128, NT], F32)
        nc.vector.tensor_scalar_mul(rc03, rcol, inv_t)

        # bounce ncol -> DRAM -> Nrep [128, N] (n_j replicated over partitions)
        nscr = dramp.tile([1, N], F32, tag="nscr")
        nflat = nscr[0]
        nc.sync.dma_start(out=bass.AP(tensor=nflat.tensor, offset=nflat.offset,
                                      ap=[[1, 128], [128, NT]]),
                          in_=ncol)
        nrep = rppool.tile([128, N], F32, tag="nrep")
        nc.gpsimd.dma_start(out=nrep, in_=_bcast_p(nflat))

        # Phase A2: X^T bf16 tiles straight from DRAM (feeds all matmuls)
        xt = []
        for dt in range(DTI):
            t = xtpool.tile([128, N], BF16)
            nc.sync.dma_start(out=t, in_=XTb[dt * 128:(dt + 1) * 128, :])
            xt.append(t)

        # Phase B interleaved with Phase C: B's PSUM eviction (threshold on
        # DVE) can lag behind the matmuls, so alternate with C's independent
        # matmul groups to keep the PE fed.  C evicts G1 raw (no d scaling):
        # both propagations use A' = diag(d) A instead, applied to the A
        # tiles in place once deg is known (valid since A is symmetric:
        # A diag(d) Y == (diag(d) A) applied via the stored [j, *] tiles).
        # Phase B: G = X X^T ; A = (G * r_i/t > n_j) (+I) ; deg fused
        # Phase C: G1 = X @ W1.T [n, h] raw bf16
        at = []
        g1 = []
        degv = bvec.tile([128, 2 * NT], F32)
        for it in range(NT):
            a_t = apool.tile([128, N], BF16)
            at.append(a_t)
            for jh in range(2):
                ps = psA.tile([128, 512], F32)
                for dt in range(DTI):
                    nc.tensor.matmul(ps, lhsT=xt[dt][:, it * 128:(it + 1) * 128],
                                     rhs=xt[dt][:, jh * 512:(jh + 1) * 512],
                                     start=(dt == 0), stop=(dt == DTI - 1))
                nc.vector.scalar_tensor_tensor(
                    out=a_t[:, jh * 512:(jh + 1) * 512], in0=ps,
                    scalar=rc03[:, it:it + 1],
                    in1=nrep[:, jh * 512:(jh + 1) * 512],
                    op0=ALU.mult, op1=ALU.is_gt,
                    accum_out=degv[:, jh * NT + it:jh * NT + it + 1])
            # self-loop: diagonal block += I
            nc.gpsimd.tensor_add(out=a_t[:, it * 128:(it + 1) * 128],
                                 in0=a_t[:, it * 128:(it + 1) * 128], in1=ident)
            # C-phase group for the same row tile (raw eviction, no scale)
            psc = psB.tile([128, D_H], F32, tag="psB")
            for dt in range(DTI):
                nc.tensor.matmul(psc, lhsT=xt[dt][:, it * 128:(it + 1) * 128],
                                 rhs=w1t[dt], start=(dt == 0), stop=(dt == DTI - 1))
            y1 = y1pool.tile([128, D_H], BF16)
            nc.scalar.copy(out=y1, in_=psc)
            g1.append(y1)

        # d = (deg)^-1/2 with deg = thresh-partials + 1 (self loop)
        dsum = bvec.tile([128, NT], F32)
        nc.vector.tensor_tensor(out=dsum, in0=degv[:, 0:NT],
                                in1=degv[:, NT:2 * NT], op=ALU.add)
        sqd = bvec.tile([128, NT], F32)
        nc.scalar.activation(out=sqd, in_=dsum, func=AF.Sqrt, bias=1.0)
        dv = bvec.tile([128, NT], F32)
        nc.vector.reciprocal(out=dv, in_=sqd)

        # Drep: d replicated across partitions via DRAM bounce
        dscr = dramp.tile([1, N], F32, tag="dscr")
        dflat = dscr[0]
        nc.sync.dma_start(out=bass.AP(tensor=dflat.tensor, offset=dflat.offset,
                                      ap=[[1, 128], [128, NT]]),
                          in_=dv)
        drep = rppool.tile([128, N], F32, tag="drep")
        nc.gpsimd.dma_start(out=drep, in_=_bcast_p(dflat))

        # A' = diag(d) A, applied in place (split DVE/GpSimd)
        for jt in range(NT):
            eng = nc.vector if jt % 2 == 0 else nc.gpsimd
            eng.tensor_scalar(out=at[jt], in0=at[jt], scalar1=dv[:, jt:jt + 1],
                              scalar2=None, op0=ALU.mult)

        # Phase D: M1^T = (A' G1)^T ; H1^T = relu(d_i * M1^T + b1)
        h1t = []
        for hc in range(HC):
            h1 = h1pool.tile([128, N], BF16)
            h1t.append(h1)
            for ih in range(2):
                ps = psA.tile([128, 512], F32)
                for jt in range(NT):
                    nc.tensor.matmul(ps, lhsT=g1[jt][:, hc * 128:(hc + 1) * 128],
                                     rhs=at[jt][:, ih * 512:(ih + 1) * 512],
                                     start=(jt == 0), stop=(jt == NT - 1))
                tmp = tmppool.tile([128, 512], F32)
                nc.vector.tensor_tensor(out=tmp, in0=ps,
                                        in1=drep[:, ih * 512:(ih + 1) * 512],
                                        op=ALU.mult)
                nc.scalar.activation(out=h1[:, ih * 512:(ih + 1) * 512], in_=tmp,
                                     func=AF.Relu, bias=b1col[:, hc:hc + 1])

        # Phase E: G2 = H1 @ W2.T [i, c] raw bf16
        ys2 = []
        for it in range(NT):
            ps = psB.tile([128, D_OUT], F32, tag="psB")
            for hc in range(HC):
                nc.tensor.matmul(ps, lhsT=h1t[hc][:, it * 128:(it + 1) * 128],
                                 rhs=w2t[hc], start=(hc == 0), stop=(hc == HC - 1))
            y2 = y2pool.tile([128, D_OUT], BF16)
            nc.vector.tensor_copy(out=y2, in_=ps)
            ys2.append(y2)

        # Phase F: M2 = A' @ G2; H2 = d_i*M2 + b2; out = H2 / max(||H2||, eps)
        for it in range(NT):
            ps = psB.tile([128, D_OUT], F32, tag="psB")
            for jt in range(NT):
                nc.tensor.matmul(ps, lhsT=at[jt][:, it * 128:(it + 1) * 128],
                                 rhs=ys2[jt], start=(jt == 0), stop=(jt == NT - 1))
            h2 = h2pool.tile([128, D_OUT], F32)
            nc.vector.tensor_scalar(out=h2, in0=ps, scalar1=dv[:, it:it + 1],
                                    scalar2=None, op0=ALU.mult)
            nc.gpsimd.tensor_add(out=h2, in0=h2, in1=b2rep)
            sj2 = sqj.tile([128, D_OUT], F32, tag="sqj2")
            ssq2 = bvec.tile([128, 1], F32, tag="ssq2")
            nc.scalar.activation(out=sj2, in_=h2, func=AF.Square, accum_out=ssq2)
            nrm2 = bvec.tile([128, 1], F32, tag="nrm2")
            nc.scalar.sqrt(out=nrm2, in_=ssq2)
            cl2 = bvec.tile([128, 1], F32, tag="cl2")
            nc.vector.tensor_scalar_max(cl2, nrm2, NORM_EPS)
            inv2 = bvec.tile([128, 1], F32, tag="inv2")
            nc.vector.reciprocal(out=inv2, in_=cl2)
            o = opool.tile([128, D_OUT], F32)
            nc.scalar.activation(out=o, in_=h2, func=AF.Copy, scale=inv2)
            nc.gpsimd.dma_start(out=Yb[it * 128:(it + 1) * 128, :], in_=o)


_NC_CACHE = {}


def _get_nc(n_batches: int = BPC):
    if n_batches not in _NC_CACHE:
        _NC_CACHE[n_batches] = build(n_batches)
    return _NC_CACHE[n_batches]


def make_in_maps(X, W1, b1, W2, b2, bpc: int = BPC):
    X = np.asarray(X, dtype=np.float32)
    Xb16 = np.ascontiguousarray(X.astype(ml_dtypes.bfloat16))
    XTb16 = np.ascontiguousarray(Xb16.transpose(0, 2, 1))
    W1T = np.ascontiguousarray(
        np.asarray(W1, dtype=np.float32).T.astype(ml_dtypes.bfloat16))
    W2T = np.ascontiguousarray(
        np.asarray(W2, dtype=np.float32).T.astype(ml_dtypes.bfloat16))
    b1 = np.ascontiguousarray(np.asarray(b1, dtype=np.float32))
    b2 = np.ascontiguousarray(np.asarray(b2, dtype=np.float32))
    return [
        {"Xb16": Xb16[c * bpc:(c + 1) * bpc], "XT": XTb16[c * bpc:(c + 1) * bpc],
         "W1T": W1T, "b1": b1, "W2T": W2T, "b2": b2}
        for c in range(len(X) // bpc)
    ]


def kernel(X, W1, b1, W2, b2):
    nc = _get_nc()
    in_maps = make_in_maps(X, W1, b1, W2, b2)
    res = run_bass_kernel_spmd(nc, in_maps, core_ids=list(range(N_CORES)))
    return np.concatenate([r["Y"] for r in res.results], axis=0)


# revision 19
# speedup vs baseline: 1.5795x; 1.5795x over previous
"""BatchedGCN Trainium2 kernel (v3).

Per graph (batch element):
  norms_i = ||X_i||;  A = (X@X.T > 0.3*n_i*n_j) + I ; deg = rowsum(A); d = deg^-1/2
  H1 = relu(diag(d) A diag(d) (X @ W1.T) + b1)
  H2 = diag(d) A diag(d) (H1 @ W2.T) + b2
  out = H2 / max(||H2_row||, 1e-12)

(The cosine threshold is applied in un-normalized form:
 Xn_i . Xn_j > t  <=>  (X_i . X_j) * (1/max(n_i,eps)) / t > n_j — exact up
 to fp rounding; the diag(norm) factor relating X to Xn cancels against
 the un-normalized X used in the first linear layer.)

Sharding: data-parallel over B=32 across 8 cores (4 graphs each);
weights replicated.  Host-side layout prep: X is fed both natural and
transposed, pre-cast to bf16 (matmul compute dtype); W1^T/W2^T likewise.
All matmuls bf16 with fp32 PSUM accumulation; A is exact {0,1,2} bf16.
"""

from contextlib import ExitStack

import ml_dtypes
import numpy as np

import concourse.bass as bass
import concourse.mybir as mybir
import concourse.tile as tile
from concourse import bacc
from concourse.bass_utils import run_bass_kernel_spmd
from concourse.masks import make_identity

B, N, D_IN, D_H, D_OUT = 32, 1024, 768, 256, 128
N_CORES = 8
BPC = B // N_CORES          # graphs per core
NT = N // 128               # 8 row tiles
DTI = D_IN // 128           # 6 input-dim tiles
HC = D_H // 128             # 2 hidden chunks
F32 = mybir.dt.float32
BF16 = mybir.dt.bfloat16

KNN_THRESHOLD = 0.3
COS_EPS = 1e-8
NORM_EPS = 1e-12
ALU = mybir.AluOpType
AF = mybir.ActivationFunctionType


def build(n_batches: int = BPC):
    nc = bacc.Bacc("TRN2", debug=False, num_devices=N_CORES)
    Xb16 = nc.dram_tensor("Xb16", [n_batches, N, D_IN], BF16, kind="ExternalInput")
    XT = nc.dram_tensor("XT", [n_batches, D_IN, N], BF16, kind="ExternalInput")
    W1T = nc.dram_tensor("W1T", [D_IN, D_H], BF16, kind="ExternalInput")
    b1 = nc.dram_tensor("b1", [D_H], F32, kind="ExternalInput")
    W2T = nc.dram_tensor("W2T", [D_H, D_OUT], BF16, kind="ExternalInput")
    b2 = nc.dram_tensor("b2", [D_OUT], F32, kind="ExternalInput")
    Y = nc.dram_tensor("Y", [n_batches, N, D_OUT], F32, kind="ExternalOutput")
    with tile.TileContext(nc) as tc, ExitStack() as ctx:
        _body(ctx, tc, Xb16.ap(), XT.ap(), W1T.ap(), b1.ap(), W2T.ap(), b2.ap(),
              Y.ap(), n_batches)
    nc.compile()
    return nc


def _bcast_p(ap: bass.AP, parts: int = 128) -> bass.AP:
    """Broadcast a DRAM AP across `parts` partitions (partition-stride 0)."""
    return bass.AP(tensor=ap.tensor, offset=ap.offset, ap=[[0, parts]] + list(ap.ap))


def _body(ctx, tc, X, XT, W1T, b1, W2T, b2, Y, n_batches):
    nc = tc.nc

    singles = ctx.enter_context(tc.tile_pool(name="singles", bufs=1))
    xpool = ctx.enter_context(tc.tile_pool(name="xpool", bufs=3))
    sqj = ctx.enter_context(tc.tile_pool(name="sqj", bufs=2))
    xtpool = ctx.enter_context(tc.tile_pool(name="xtpool", bufs=2 * DTI))
    apool = ctx.enter_context(tc.tile_pool(name="apool", bufs=2 * NT))
    bvec = ctx.enter_context(tc.tile_pool(name="bvec", bufs=2))
    y1pool = ctx.enter_context(tc.tile_pool(name="y1pool", bufs=2 * NT))
    h1pool = ctx.enter_context(tc.tile_pool(name="h1pool", bufs=2 * HC))
    y2pool = ctx.enter_context(tc.tile_pool(name="y2pool", bufs=2 * NT))
    rppool = ctx.enter_context(tc.tile_pool(name="rppool", bufs=2))
    tmppool = ctx.enter_context(tc.tile_pool(name="tmppool", bufs=3))
    h2pool = ctx.enter_context(tc.tile_pool(name="h2pool", bufs=3))
    opool = ctx.enter_context(tc.tile_pool(name="opool", bufs=3))
    psA = ctx.enter_context(tc.tile_pool(name="psA", bufs=4, space="PSUM"))
    psB = ctx.enter_context(tc.tile_pool(name="psB", bufs=4, space="PSUM"))
    dramp = ctx.enter_context(tc.tile_pool(name="dramp", bufs=2, space="DRAM"))

    # ---- one-time constants (all plain loads, no prep chains) ---------------
    ident = singles.tile([128, 128], BF16)
    make_identity(nc, ident)

    b1col = singles.tile([128, HC], F32)
    nc.sync.dma_start(out=b1col, in_=bass.AP(tensor=b1.tensor, offset=b1.offset,
                                             ap=[[1, 128], [128, HC]]))
    b2rep = singles.tile([128, D_OUT], F32)
    nc.gpsimd.dma_start(out=b2rep, in_=_bcast_p(b2))

    w1t = []
    for dt in range(DTI):
        t = singles.tile([128, D_H], BF16, tag=f"w1t{dt}")
        nc.sync.dma_start(out=t, in_=W1T[dt * 128:(dt + 1) * 128, :])
        w1t.append(t)
    w2t = []
    for k in range(HC):
        t = singles.tile([128, D_OUT], BF16, tag=f"w2t{k}")
        nc.sync.dma_start(out=t, in_=W2T[k * 128:(k + 1) * 128, :])
        w2t.append(t)

    inv_t = 1.0 / KNN_THRESHOLD

    # ---- per-graph pipeline -------------------------------------------------
    for bi in range(n_batches):
        Xb = X[bi]
        XTb = XT[bi]
        Yb = Y[bi]

        # Phase A1 first: row norms from X natural layout — this feeds the
        # threshold bound (Nrep), the longest per-batch latency chain.
        ssqv = bvec.tile([128, NT], F32)
        for nt in range(NT):
            xf = xpool.tile([128, D_IN], BF16, tag="xf")
            nc.sync.dma_start(out=xf, in_=Xb[nt * 128:(nt + 1) * 128, :])
            sj = sqj.tile([128, D_IN], F32)
            nc.scalar.activation(out=sj, in_=xf, func=AF.Square,
                                 accum_out=ssqv[:, nt:nt + 1])
        ncol = bvec.tile([128, NT], F32)
        nc.scalar.sqrt(out=ncol, in_=ssqv)
        nclamp = bvec.tile([128, NT], F32)
        nc.vector.tensor_scalar_max(nclamp, ncol, COS_EPS)
        rcol = bvec.tile([128, NT], F32)
        nc.vector.reciprocal(out=rcol, in_=nclamp)
        rc03 = bvec.tile([128, NT], F32)
        nc.vector.tensor_scalar_mul(rc03, rcol, inv_t)

        # bounce ncol -> DRAM -> Nrep [128, N] (n_j replicated over partitions)
        nscr = dramp.tile([1, N], F32, tag="nscr")
        nflat = nscr[0]
        nc.sync.dma_start(out=bass.AP(tensor=nflat.tensor, offset=nflat.offset,
                                      ap=[[1, 128], [128, NT]]),
                          in_=ncol)
        nrep = rppool.tile([128, N], F32, tag="nrep")
        nc.gpsimd.dma_start(out=nrep, in_=_bcast_p(nflat))

        # Phase A2: X^T bf16 tiles straight from DRAM (feeds all matmuls)
        xt = []
        for dt in range(DTI):
            t = xtpool.tile([128, N], BF16)
            nc.sync.dma_start(out=t, in_=XTb[dt * 128:(dt + 1) * 128, :])
            xt.append(t)

        # Phase B interleaved with part of Phase C: B's PSUM eviction
        # (threshold on DVE) can lag behind the matmuls, so alternate with
        # C's independent matmul groups to keep the PE fed.
        # Phase B: G = X X^T ; A = (G * r_i/t > n_j) (+I) ; deg fused
        # Phase C: G1 = X @ W1.T [n, h]; evicted as Ys1 = d * G1 (bf16)
        N_C_EARLY = 4  # C groups parked in psB banks while B runs
        at = []
        cps = []
        degv = bvec.tile([128, 2 * NT], F32)
        for it in range(NT):
            a_t = apool.tile([128, N], BF16)
            at.append(a_t)
            for jh in range(2):
                ps = psA.tile([128, 512], F32)
                for dt in range(DTI):
                    nc.tensor.matmul(ps, lhsT=xt[dt][:, it * 128:(it + 1) * 128],
                                     rhs=xt[dt][:, jh * 512:(jh + 1) * 512],
                                     start=(dt == 0), stop=(dt == DTI - 1))
                nc.vector.scalar_tensor_tensor(
                    out=a_t[:, jh * 512:(jh + 1) * 512], in0=ps,
                    scalar=rc03[:, it:it + 1],
                    in1=nrep[:, jh * 512:(jh + 1) * 512],
                    op0=ALU.mult, op1=ALU.is_gt,
                    accum_out=degv[:, jh * NT + it:jh * NT + it + 1])
            # self-loop: diagonal block += I
            nc.gpsimd.tensor_add(out=a_t[:, it * 128:(it + 1) * 128],
                                 in0=a_t[:, it * 128:(it + 1) * 128], in1=ident)
            if it < N_C_EARLY:
                psc = psB.tile([128, D_H], F32, tag="psB")
                for dt in range(DTI):
                    nc.tensor.matmul(psc, lhsT=xt[dt][:, it * 128:(it + 1) * 128],
                                     rhs=w1t[dt], start=(dt == 0),
                                     stop=(dt == DTI - 1))
                cps.append(psc)

        # d = (deg)^-1/2 with deg = thresh-partials + 1 (self loop)
        dsum = bvec.tile([128, NT], F32)
        nc.vector.tensor_tensor(out=dsum, in0=degv[:, 0:NT],
                                in1=degv[:, NT:2 * NT], op=ALU.add)
        sqd = bvec.tile([128, NT], F32)
        nc.scalar.activation(out=sqd, in_=dsum, func=AF.Sqrt, bias=1.0)
        dv = bvec.tile([128, NT], F32)
        nc.vector.reciprocal(out=dv, in_=sqd)

        # Drep: d replicated across partitions via DRAM bounce
        dscr = dramp.tile([1, N], F32, tag="dscr")
        dflat = dscr[0]
        nc.sync.dma_start(out=bass.AP(tensor=dflat.tensor, offset=dflat.offset,
                                      ap=[[1, 128], [128, NT]]),
                          in_=dv)
        drep = rppool.tile([128, N], F32, tag="drep")
        nc.gpsimd.dma_start(out=drep, in_=_bcast_p(dflat))

        # finish Phase C: evict the parked groups, run the rest
        ys1 = []
        for it in range(NT):
            if it < N_C_EARLY:
                psc = cps[it]
            else:
                psc = psB.tile([128, D_H], F32, tag="psB")
                for dt in range(DTI):
                    nc.tensor.matmul(psc, lhsT=xt[dt][:, it * 128:(it + 1) * 128],
                                     rhs=w1t[dt], start=(dt == 0),
                                     stop=(dt == DTI - 1))
            y1 = y1pool.tile([128, D_H], BF16)
            nc.scalar.activation(out=y1, in_=psc, func=AF.Copy,
                                 scale=dv[:, it:it + 1])
            ys1.append(y1)

        # Phase D: M1^T = (A diag(d) G1)^T ; H1^T = relu(d_i * M1^T + b1)
        h1t = []
        for hc in range(HC):
            h1 = h1pool.tile([128, N], BF16)
            h1t.append(h1)
            for ih in range(2):
                ps = psA.tile([128, 512], F32)
                for jt in range(NT):
                    nc.tensor.matmul(ps, lhsT=ys1[jt][:, hc * 128:(hc + 1) * 128],
                                     rhs=at[jt][:, ih * 512:(ih + 1) * 512],
                                     start=(jt == 0), stop=(jt == NT - 1))
                tmp = tmppool.tile([128, 512], F32)
                nc.vector.tensor_tensor(out=tmp, in0=ps,
                                        in1=drep[:, ih * 512:(ih + 1) * 512],
                                        op=ALU.mult)
                nc.scalar.activation(out=h1[:, ih * 512:(ih + 1) * 512], in_=tmp,
                                     func=AF.Relu, bias=b1col[:, hc:hc + 1])

        # Phase E: G2 = H1 @ W2.T [i, c]; evict scaled by d -> Ys2 bf16
        ys2 = []
        for it in range(NT):
            ps = psB.tile([128, D_OUT], F32, tag="psB")
            for hc in range(HC):
                nc.tensor.matmul(ps, lhsT=h1t[hc][:, it * 128:(it + 1) * 128],
                                 rhs=w2t[hc], start=(hc == 0), stop=(hc == HC - 1))
            y2 = y2pool.tile([128, D_OUT], BF16)
            nc.vector.tensor_scalar(out=y2, in0=ps, scalar1=dv[:, it:it + 1],
                                    scalar2=None, op0=ALU.mult)
            ys2.append(y2)

        # Phase F: M2 = A @ Ys2; H2 = d_i*M2 + b2; out = H2 / max(||H2||, eps)
        for it in range(NT):
            ps = psB.tile([128, D_OUT], F32, tag="psB")
            for jt in range(NT):
                nc.tensor.matmul(ps, lhsT=at[jt][:, it * 128:(it + 1) * 128],
                                 rhs=ys2[jt], start=(jt == 0), stop=(jt == NT - 1))
            h2 = h2pool.tile([128, D_OUT], F32)
            nc.vector.tensor_scalar(out=h2, in0=ps, scalar1=dv[:, it:it + 1],
                                    scalar2=None, op0=ALU.mult)
            nc.gpsimd.tensor_add(out=h2, in0=h2, in1=b2rep)
            sj2 = sqj.tile([128, D_OUT], F32, tag="sqj2")
            ssq2 = bvec.tile([128, 1], F32, tag="ssq2")
            nc.scalar.activation(out=sj2, in_=h2, func=AF.Square, accum_out=ssq2)
            nrm2 = bvec.tile([128, 1], F32, tag="nrm2")
            nc.scalar.sqrt(out=nrm2, in_=ssq2)
            cl2 = bvec.tile([128, 1], F32, tag="cl2")
            nc.vector.tensor_scalar_max(cl2, nrm2, NORM_EPS)
            inv2 = bvec.tile([128, 1], F32, tag="inv2")
            nc.vector.reciprocal(out=inv2, in_=cl2)
            o = opool.tile([128, D_OUT], F32)
            nc.scalar.activation(out=o, in_=h2, func=AF.Copy, scale=inv2)
            nc.gpsimd.dma_start(out=Yb[it * 128:(it + 1) * 128, :], in_=o)


_NC_CACHE = {}


def _get_nc(n_batches: int = BPC):
    if n_batches not in _NC_CACHE:
        _NC_CACHE[n_batches] = build(n_batches)
    return _NC_CACHE[n_batches]


def make_in_maps(X, W1, b1, W2, b2, bpc: int = BPC):
    X = np.asarray(X, dtype=np.float32)
    Xb16 = np.ascontiguousarray(X.astype(ml_dtypes.bfloat16))
    XTb16 = np.ascontiguousarray(Xb16.transpose(0, 2, 1))
    W1T = np.ascontiguousarray(
        np.asarray(W1, dtype=np.float32).T.astype(ml_dtypes.bfloat16))
    W2T = np.ascontiguousarray(
        np.asarray(W2, dtype=np.float32).T.astype(ml_dtypes.bfloat16))
    b1 = np.ascontiguousarray(np.asarray(b1, dtype=np.float32))
    b2 = np.ascontiguousarray(np.asarray(b2, dtype=np.float32))
    return [
        {"Xb16": Xb16[c * bpc:(c + 1) * bpc], "XT": XTb16[c * bpc:(c + 1) * bpc],
         "W1T": W1T, "b1": b1, "W2T": W2T, "b2": b2}
        for c in range(len(X) // bpc)
    ]


def kernel(X, W1, b1, W2, b2):
    nc = _get_nc()
    in_maps = make_in_maps(X, W1, b1, W2, b2)
    res = run_bass_kernel_spmd(nc, in_maps, core_ids=list(range(N_CORES)))
    return np.concatenate([r["Y"] for r in res.results], axis=0)


# revision 21
# speedup vs baseline: 1.7312x; 1.0960x over previous
"""BatchedGCN Trainium2 kernel (v6).

Per graph (batch element):
  norms_i = ||X_i||;  A = (X@X.T > 0.3*n_i*n_j) + I ; deg = rowsum(A); d = deg^-1/2
  H1 = relu(diag(d) A diag(d) (X @ W1.T) + b1)
  H2 = diag(d) A diag(d) (H1 @ W2.T) + b2
  out = H2 / max(||H2_row||, 1e-12)

(The cosine threshold is applied in un-normalized form:
 Xn_i . Xn_j > t  <=>  (X_i . X_j) * (1/max(n_i,eps)) / t > n_j — exact up
 to fp rounding; the diag(norm) factor relating X to Xn cancels against
 the un-normalized X used in the first linear layer.)

Sharding: data-parallel over B=32 across 8 cores (4 graphs each);
weights replicated.  Host-side layout prep: X is fed both natural and
transposed, pre-cast to bf16 (matmul compute dtype); W1^T/W2^T likewise.
All matmuls bf16 with fp32 PSUM accumulation; A is exact {0,1,2} bf16.

Graphs are processed in software-pipelined PAIRS: every phase's matmul
groups are emitted alternating between the two graphs of a pair, so the
TensorEngine's static schedule always has independent work adjacent to
any dependency stall (threshold eviction lag, deg -> d -> DRAM-bounce
latency), keeping the PE dense and HAM-warm.
"""

from contextlib import ExitStack

import ml_dtypes
import numpy as np

import concourse.bass as bass
import concourse.mybir as mybir
import concourse.tile as tile
from concourse import bacc
from concourse.bass_utils import run_bass_kernel_spmd
from concourse.masks import make_identity

B, N, D_IN, D_H, D_OUT = 32, 1024, 768, 256, 128
N_CORES = 8
BPC = B // N_CORES          # graphs per core
NT = N // 128               # 8 row tiles
DTI = D_IN // 128           # 6 input-dim tiles
HC = D_H // 128             # 2 hidden chunks
F32 = mybir.dt.float32
BF16 = mybir.dt.bfloat16

KNN_THRESHOLD = 0.3
COS_EPS = 1e-8
NORM_EPS = 1e-12
ALU = mybir.AluOpType
AF = mybir.ActivationFunctionType


def build(n_batches: int = BPC):
    nc = bacc.Bacc("TRN2", debug=False, num_devices=N_CORES)
    Xb16 = nc.dram_tensor("Xb16", [n_batches, N, D_IN], BF16, kind="ExternalInput")
    XT = nc.dram_tensor("XT", [n_batches, D_IN, N], BF16, kind="ExternalInput")
    W1T = nc.dram_tensor("W1T", [D_IN, D_H], BF16, kind="ExternalInput")
    b1 = nc.dram_tensor("b1", [D_H], F32, kind="ExternalInput")
    W2T = nc.dram_tensor("W2T", [D_H, D_OUT], BF16, kind="ExternalInput")
    b2 = nc.dram_tensor("b2", [D_OUT], F32, kind="ExternalInput")
    Y = nc.dram_tensor("Y", [n_batches, N, D_OUT], F32, kind="ExternalOutput")
    with tile.TileContext(nc) as tc, ExitStack() as ctx:
        _body(ctx, tc, Xb16.ap(), XT.ap(), W1T.ap(), b1.ap(), W2T.ap(), b2.ap(),
              Y.ap(), n_batches)
    nc.compile()
    return nc


def _bcast_p(ap: bass.AP, parts: int = 128) -> bass.AP:
    """Broadcast a DRAM AP across `parts` partitions (partition-stride 0)."""
    return bass.AP(tensor=ap.tensor, offset=ap.offset, ap=[[0, parts]] + list(ap.ap))


class _GraphState:
    """Per-graph SBUF tiles threaded between pipeline phases."""
    __slots__ = ("Xb", "XTb", "Yb", "xt", "at", "ys1", "ys2", "h1t",
                 "rc03", "nrep", "degv", "dv", "drep")


def _body(ctx, tc, X, XT, W1T, b1, W2T, b2, Y, n_batches):
    nc = tc.nc

    singles = ctx.enter_context(tc.tile_pool(name="singles", bufs=1))
    xpool = ctx.enter_context(tc.tile_pool(name="xpool", bufs=3))
    sqj = ctx.enter_context(tc.tile_pool(name="sqj", bufs=2))
    xtpool = ctx.enter_context(tc.tile_pool(name="xtpool", bufs=2 * DTI))
    apool = ctx.enter_context(tc.tile_pool(name="apool", bufs=2 * NT))
    bvec = ctx.enter_context(tc.tile_pool(name="bvec", bufs=4))
    y1pool = ctx.enter_context(tc.tile_pool(name="y1pool", bufs=2 * NT))
    h1pool = ctx.enter_context(tc.tile_pool(name="h1pool", bufs=2 * HC))
    y2pool = ctx.enter_context(tc.tile_pool(name="y2pool", bufs=2 * NT))
    rppool = ctx.enter_context(tc.tile_pool(name="rppool", bufs=2))
    tmppool = ctx.enter_context(tc.tile_pool(name="tmppool", bufs=3))
    h2pool = ctx.enter_context(tc.tile_pool(name="h2pool", bufs=3))
    opool = ctx.enter_context(tc.tile_pool(name="opool", bufs=3))
    psA = ctx.enter_context(tc.tile_pool(name="psA", bufs=4, space="PSUM"))
    psB = ctx.enter_context(tc.tile_pool(name="psB", bufs=4, space="PSUM"))
    dramp = ctx.enter_context(tc.tile_pool(name="dramp", bufs=4, space="DRAM"))

    # ---- one-time constants (all plain loads, no prep chains) ---------------
    ident = singles.tile([128, 128], BF16)
    make_identity(nc, ident)

    b1col = singles.tile([128, HC], F32)
    nc.sync.dma_start(out=b1col, in_=bass.AP(tensor=b1.tensor, offset=b1.offset,
                                             ap=[[1, 128], [128, HC]]))
    b2rep = singles.tile([128, D_OUT], F32)
    nc.gpsimd.dma_start(out=b2rep, in_=_bcast_p(b2))

    w1t = []
    for dt in range(DTI):
        t = singles.tile([128, D_H], BF16, tag=f"w1t{dt}")
        nc.sync.dma_start(out=t, in_=W1T[dt * 128:(dt + 1) * 128, :])
        w1t.append(t)
    w2t = []
    for k in range(HC):
        t = singles.tile([128, D_OUT], BF16, tag=f"w2t{k}")
        nc.sync.dma_start(out=t, in_=W2T[k * 128:(k + 1) * 128, :])
        w2t.append(t)

    inv_t = 1.0 / KNN_THRESHOLD

    # ---- per-phase emitters -------------------------------------------------
    def phase_a(g: _GraphState):
        # A1: row norms from X natural layout -> Nrep bound chain
        ssqv = bvec.tile([128, NT], F32, tag="ssqv")
        for nt in range(NT):
            xf = xpool.tile([128, D_IN], BF16, tag="xf")
            nc.sync.dma_start(out=xf, in_=g.Xb[nt * 128:(nt + 1) * 128, :])
            sj = sqj.tile([128, D_IN], F32)
            nc.scalar.activation(out=sj, in_=xf, func=AF.Square,
                                 accum_out=ssqv[:, nt:nt + 1])
        ncol = bvec.tile([128, NT], F32, tag="ncol")
        nc.scalar.sqrt(out=ncol, in_=ssqv)
        nclamp = bvec.tile([128, NT], F32, tag="nclamp")
        nc.vector.tensor_scalar_max(nclamp, ncol, COS_EPS)
        rcol = bvec.tile([128, NT], F32, tag="rcol")
        nc.vector.reciprocal(out=rcol, in_=nclamp)
        g.rc03 = bvec.tile([128, NT], F32, tag="rc03")
        nc.vector.tensor_scalar_mul(g.rc03, rcol, inv_t)

        nscr = dramp.tile([1, N], F32, tag="nscr")
        nflat = nscr[0]
        nc.sync.dma_start(out=bass.AP(tensor=nflat.tensor, offset=nflat.offset,
                                      ap=[[1, 128], [128, NT]]),
                          in_=ncol)
        g.nrep = rppool.tile([128, N], F32, tag="nrep")
        nc.gpsimd.dma_start(out=g.nrep, in_=_bcast_p(nflat))

        # A2: X^T bf16 tiles straight from DRAM
        g.xt = []
        for dt in range(DTI):
            t = xtpool.tile([128, N], BF16, tag="xt")
            nc.sync.dma_start(out=t, in_=g.XTb[dt * 128:(dt + 1) * 128, :])
            g.xt.append(t)
        g.at = []
        g.ys1 = []
        g.ys2 = []
        g.h1t = []
        g.degv = bvec.tile([128, 2 * NT], F32, tag="degv")

    def phase_b_group(g: _GraphState, it: int):
        # one row tile of G = X X^T -> threshold -> A row tile (+ self loop)
        a_t = apool.tile([128, N], BF16, tag="a_t")
        g.at.append(a_t)
        for jh in range(2):
            ps = psA.tile([128, 512], F32, tag="psA")
            for dt in range(DTI):
                nc.tensor.matmul(ps, lhsT=g.xt[dt][:, it * 128:(it + 1) * 128],
                                 rhs=g.xt[dt][:, jh * 512:(jh + 1) * 512],
                                 start=(dt == 0), stop=(dt == DTI - 1))
            nc.vector.scalar_tensor_tensor(
                out=a_t[:, jh * 512:(jh + 1) * 512], in0=ps,
                scalar=g.rc03[:, it:it + 1],
                in1=g.nrep[:, jh * 512:(jh + 1) * 512],
                op0=ALU.mult, op1=ALU.is_gt,
                accum_out=g.degv[:, jh * NT + it:jh * NT + it + 1])
        nc.gpsimd.tensor_add(out=a_t[:, it * 128:(it + 1) * 128],
                             in0=a_t[:, it * 128:(it + 1) * 128], in1=ident)

    def phase_dpipe(g: _GraphState):
        # deg -> d = deg^-1/2 -> Drep bounce
        dsum = bvec.tile([128, NT], F32, tag="dsum")
        nc.vector.tensor_tensor(out=dsum, in0=g.degv[:, 0:NT],
                                in1=g.degv[:, NT:2 * NT], op=ALU.add)
        sqd = bvec.tile([128, NT], F32, tag="sqd")
        nc.scalar.activation(out=sqd, in_=dsum, func=AF.Sqrt, bias=1.0)
        g.dv = bvec.tile([128, NT], F32, tag="dv")
        nc.vector.reciprocal(out=g.dv, in_=sqd)

        dscr = dramp.tile([1, N], F32, tag="dscr")
        dflat = dscr[0]
        nc.sync.dma_start(out=bass.AP(tensor=dflat.tensor, offset=dflat.offset,
                                      ap=[[1, 128], [128, NT]]),
                          in_=g.dv)
        g.drep = rppool.tile([128, N], F32, tag="drep")
        nc.gpsimd.dma_start(out=g.drep, in_=_bcast_p(dflat))

    def phase_c_group(g: _GraphState, it: int):
        ps = psB.tile([128, D_H], F32, tag="psB")
        for dt in range(DTI):
            nc.tensor.matmul(ps, lhsT=g.xt[dt][:, it * 128:(it + 1) * 128],
                             rhs=w1t[dt], start=(dt == 0), stop=(dt == DTI - 1))
        y1 = y1pool.tile([128, D_H], BF16, tag="y1")
        nc.scalar.activation(out=y1, in_=ps, func=AF.Copy,
                             scale=g.dv[:, it:it + 1])
        g.ys1.append(y1)

    def phase_d_group(g: _GraphState, hc: int, ih: int):
        if ih == 0:
            g.h1t.append(h1pool.tile([128, N], BF16, tag="h1", name="h1"))
        h1 = g.h1t[hc]
        ps = psA.tile([128, 512], F32, tag="psA")
        for jt in range(NT):
            nc.tensor.matmul(ps, lhsT=g.ys1[jt][:, hc * 128:(hc + 1) * 128],
                             rhs=g.at[jt][:, ih * 512:(ih + 1) * 512],
                             start=(jt == 0), stop=(jt == NT - 1))
        tmp = tmppool.tile([128, 512], F32, tag="tmp")
        nc.vector.tensor_tensor(out=tmp, in0=ps,
                                in1=g.drep[:, ih * 512:(ih + 1) * 512],
                                op=ALU.mult)
        nc.scalar.activation(out=h1[:, ih * 512:(ih + 1) * 512], in_=tmp,
                             func=AF.Relu, bias=b1col[:, hc:hc + 1])

    def phase_e_group(g: _GraphState, it: int):
        ps = psB.tile([128, D_OUT], F32, tag="psB")
        for hc in range(HC):
            nc.tensor.matmul(ps, lhsT=g.h1t[hc][:, it * 128:(it + 1) * 128],
                             rhs=w2t[hc], start=(hc == 0), stop=(hc == HC - 1))
        y2 = y2pool.tile([128, D_OUT], BF16, tag="y2")
        nc.vector.tensor_scalar(out=y2, in0=ps, scalar1=g.dv[:, it:it + 1],
                                scalar2=None, op0=ALU.mult)
        g.ys2.append(y2)

    def phase_f_group(g: _GraphState, it: int):
        ps = psB.tile([128, D_OUT], F32, tag="psB")
        for jt in range(NT):
            nc.tensor.matmul(ps, lhsT=g.at[jt][:, it * 128:(it + 1) * 128],
                             rhs=g.ys2[jt], start=(jt == 0), stop=(jt == NT - 1))
        h2 = h2pool.tile([128, D_OUT], F32, tag="h2")
        nc.vector.tensor_scalar(out=h2, in0=ps, scalar1=g.dv[:, it:it + 1],
                                scalar2=None, op0=ALU.mult)
        nc.gpsimd.tensor_add(out=h2, in0=h2, in1=b2rep)
        sj2 = sqj.tile([128, D_OUT], F32, tag="sqj2")
        ssq2 = bvec.tile([128, 1], F32, tag="ssq2")
        nc.scalar.activation(out=sj2, in_=h2, func=AF.Square, accum_out=ssq2)
        nrm2 = bvec.tile([128, 1], F32, tag="nrm2")
        nc.scalar.sqrt(out=nrm2, in_=ssq2)
        cl2 = bvec.tile([128, 1], F32, tag="cl2")
        nc.vector.tensor_scalar_max(cl2, nrm2, NORM_EPS)
        inv2 = bvec.tile([128, 1], F32, tag="inv2")
        nc.vector.reciprocal(out=inv2, in_=cl2)
        o = opool.tile([128, D_OUT], F32, tag="o")
        nc.scalar.activation(out=o, in_=h2, func=AF.Copy, scale=inv2)
        nc.gpsimd.dma_start(out=g.Yb[it * 128:(it + 1) * 128, :], in_=o)

    # ---- pair-pipelined driver ----------------------------------------------
    for b0 in range(0, n_batches, 2):
        pair = []
        for bi in (b0, b0 + 1):
            if bi >= n_batches:
                break
            g = _GraphState()
            g.Xb, g.XTb, g.Yb = X[bi], XT[bi], Y[bi]
            pair.append(g)

        for g in pair:
            phase_a(g)
        for it in range(NT):
            for g in pair:
                phase_b_group(g, it)
        for g in pair:
            phase_dpipe(g)
        for it in range(NT):
            for g in pair:
                phase_c_group(g, it)
        for hc in range(HC):
            for ih in range(2):
                for g in pair:
                    phase_d_group(g, hc, ih)
        for it in range(NT):
            for g in pair:
                phase_e_group(g, it)
        for it in range(NT):
            for g in pair:
                phase_f_group(g, it)


_NC_CACHE = {}


def _get_nc(n_batches: int = BPC):
    if n_batches not in _NC_CACHE:
        _NC_CACHE[n_batches] = build(n_batches)
    return _NC_CACHE[n_batches]


def make_in_maps(X, W1, b1, W2, b2, bpc: int = BPC):
    X = np.asarray(X, dtype=np.float32)
    Xb16 = np.ascontiguousarray(X.astype(ml_dtypes.bfloat16))
    XTb16 = np.ascontiguousarray(Xb16.transpose(0, 2, 1))
    W1T = np.ascontiguousarray(
        np.asarray(W1, dtype=np.float32).T.astype(ml_dtypes.bfloat16))
    W2T = np.ascontiguousarray(
        np.asarray(W2, dtype=np.float32).T.astype(ml_dtypes.bfloat16))
    b1 = np.ascontiguousarray(np.asarray(b1, dtype=np.float32))
    b2 = np.ascontiguousarray(np.asarray(b2, dtype=np.float32))
    return [
        {"Xb16": Xb16[c * bpc:(c + 1) * bpc], "XT": XTb16[c * bpc:(c + 1) * bpc],
         "W1T": W1T, "b1": b1, "W2T": W2T, "b2": b2}
        for c in range(len(X) // bpc)
    ]


def kernel(X, W1, b1, W2, b2):
    nc = _get_nc()
    in_maps = make_in_maps(X, W1, b1, W2, b2)
    res = run_bass_kernel_spmd(nc, in_maps, core_ids=list(range(N_CORES)))
    return np.concatenate([r["Y"] for r in res.results], axis=0)
